# revision 6
# baseline (speedup 1.0000x reference)
"""AGCN (2-layer GCN) forward on 8 TRN2 NeuronCores.

Math (reference):
    agg(h)[d] = sum_{(s,d) in E+selfloops} dinv[s]*dinv[d] * h[s]
    out = relu(agg(relu(agg(x) @ W1 + b1) @ W2) + b2)

Distribution: nodes row-sharded 8 ways by dst (6250/core + pad -> 6272).
Per-core, per 128-node dst block, the normalized adjacency slice is encoded
as host-built "selection" matrices S.T [128 edge-slots, 128 dst] (entry =
summed norm of edges from that slot's source to that dst column).
Aggregation is then:  dma_gather of source rows (bf16, SWDGE) + TensorE
matmuls accumulating in f32 PSUM -- no scatter, no races.

- Self-loop contributions use a contiguous DMA of the block's own rows
  (slot = node) instead of gather slots: saves ~6% of gather traffic.
- Sources deduplicated per (block): each distinct src gets one slot; its
  S.T row carries one entry per edge (possibly several dst columns).
- int16 gather indices reach 32767 < 50000, so gather sources split lo/hi
  at node 25000 (identical split for the z layout: 25088 = 4*6272).
- Layer 1 computes s1.T feature-major (lhsT=gathered, rhs=S.T), chains
  W1/W2 in feature-major, transposes back via TensorE, stores z bf16.
  One AllGather shares z; layer 2 aggregates with the same S tiles
  (lhsT=S.T, rhs=gathered z) giving node-major output.
"""
import sys

for _p in ("/opt/trn_rl_repo", "/root/.axon_site/_ro/trn_rl_repo"):
    if _p not in sys.path:
        sys.path.append(_p)

import ml_dtypes
import numpy as np

from concourse import bacc, mybir, tile
from concourse.bass_utils import run_bass_kernel_spmd

BF16 = ml_dtypes.bfloat16

N = 50000
D = 128
H = 512
NCORES = 8
R = N // NCORES          # 6250 real rows per core
NBLK = (R + 127) // 128  # 49
RPAD = NBLK * 128        # 6272
GB = 7                   # blocks per gather group
NGRP = NBLK // GB        # 7
SPLIT = N // 2           # lo/hi gather split (src < 25000)
ZSPLIT = (NCORES // 2) * RPAD  # 25088
OP_IDX = 512            # max indices per dma_gather (SWDGE ring limit)


def _prep(x, edge_index, W1, b1, W2, b2):
    src = np.asarray(edge_index[0], dtype=np.int64)
    dst = np.asarray(edge_index[1], dtype=np.int64)
    deg = np.bincount(dst, minlength=N).astype(np.float64) + 1.0  # + self loop
    dinv = 1.0 / np.sqrt(deg)
    norm = (dinv[src] * dinv[dst]).astype(np.float32)

    core = dst // R
    blk = (dst - core * R) // 128
    dcol = dst - core * R - blk * 128

    # ---- per (core, block): dedup sources, split lo/hi, count slots ----
    # slot key: (core, blk, src) -> slot id per (core, blk, half)
    percore = []
    cnt_lo = np.zeros((NCORES, NBLK), np.int64)
    cnt_hi = np.zeros((NCORES, NBLK), np.int64)
    for c in range(NCORES):
        m = core == c
        cb = blk[m]
        cs = src[m]
        cd = dcol[m]
        cn = norm[m]
        is_lo = cs < SPLIT
        # unique (blk, src) pairs per half
        info = {}
        for half, hm in (("lo", is_lo), ("hi", ~is_lo)):
            key = cb[hm] * N + cs[hm]
            uk, inv = np.unique(key, return_inverse=True)
            ub = uk // N
            us = uk % N
            # slot index within (blk, half): rank among same-blk uniques
            order = np.argsort(ub, kind="stable")
            rank = np.empty(uk.shape[0], np.int64)
            pos_in_blk = np.zeros(uk.shape[0], np.int64)
            # ub is sorted groups via order; compute per-group running index
            sb = ub[order]
            start = np.r_[True, sb[1:] != sb[:-1]]
            grp_start = np.maximum.accumulate(np.where(start, np.arange(sb.size), 0))
            pos_in_blk[order] = np.arange(sb.size) - grp_start
            info[half] = dict(
                ub=ub, us=us, slot=pos_in_blk, inv=inv,
                dcol=cd[hm], nrm=cn[hm], eblk=cb[hm],
            )
            cnts = np.bincount(ub, minlength=NBLK)
            if half == "lo":
                cnt_lo[c] = cnts
            else:
                cnt_hi[c] = cnts
        percore.append(info)

    NSL = -(-cnt_lo.max(axis=0) // 128)  # [NBLK] lo subtiles per block
    NSH = -(-cnt_hi.max(axis=0) // 128)
    NS = NSL + NSH + 1                   # + self subtile (last)
    sub_off = np.r_[0, np.cumsum(NS)]    # [NBLK+1]
    TOT_SUB = int(sub_off[-1])

    # gather stream layout per (group, half): concat of blocks' slot arrays
    # (each padded to NSL/NSH subtiles); op sizes chop stream into <=OP_IDX
    def stream_meta(ns_half):
        offs = []  # per block: subtile offset within its group's stream
        lens = []  # per group: total idx
        for g in range(NGRP):
            o = 0
            for b in range(g * GB, (g + 1) * GB):
                offs.append(o)
                o += int(ns_half[b]) * 128
            lens.append(o)
        return offs, lens

    lo_boff, lo_glen = stream_meta(NSL)
    hi_boff, hi_glen = stream_meta(NSH)

    def op_sizes(glen):
        out = []
        for L in glen:
            sizes = [OP_IDX] * (L // OP_IDX)
            if L % OP_IDX:
                sizes.append(L % OP_IDX)
            out.append(sizes)
        return out

    lo_ops = op_sizes(lo_glen)
    hi_ops = op_sizes(hi_glen)

    meta = dict(NSL=NSL, NSH=NSH, NS=NS, sub_off=sub_off, TOT_SUB=TOT_SUB,
                lo_boff=lo_boff, hi_boff=hi_boff, lo_glen=lo_glen,
                hi_glen=hi_glen, lo_ops=lo_ops, hi_ops=hi_ops)

    # ---- build per-core arrays ----
    L_lo = sum(lo_glen)
    L_hi = sum(hi_glen)
    w1_bf = np.asarray(W1, np.float32).astype(BF16)
    w2_bf = np.asarray(W2, np.float32).astype(BF16)
    b1_f = np.asarray(b1, np.float32).reshape(4, 128).T.copy()
    b2_bc = np.tile(np.asarray(b2, np.float32)[None, :], (128, 1))
    eye = np.eye(128, dtype=BF16)
    xs_bf = np.asarray(x, np.float32).astype(BF16)
    self_norm = (dinv * dinv).astype(np.float32)

    def wrap(idx):
        k = idx.shape[0]
        w = idx.reshape(k // 16, 16).T.astype(np.int16)
        return np.ascontiguousarray(np.tile(w, (8, 1)))

    in_maps = []
    for c in range(NCORES):
        info = percore[c]
        st = np.zeros((128, TOT_SUB, 128), dtype=np.float32)
        ix = {"lo": np.zeros(L_lo, np.int64), "hi": np.zeros(L_hi, np.int64)}
        iz = {"lo": np.zeros(L_lo, np.int64), "hi": np.zeros(L_hi, np.int64)}
        for half, boff, base in (("lo", lo_boff, 0), ("hi", hi_boff, SPLIT)):
            d = info[half]
            # stream position of each unique source
            g = d["ub"] // GB
            goff = np.array([sum((lo_glen if half == "lo" else hi_glen)[:gg]) for gg in g])
            bo = np.array([boff[bb] for bb in d["ub"]])
            spos = goff + bo + d["slot"]
            ix[half][spos] = d["us"] - base
            zr = (d["us"] // R) * RPAD + (d["us"] % R)
            iz[half][spos] = zr - (0 if half == "lo" else ZSPLIT)
            # S entries: edge e -> unique u = inv[e]; subtile/partition from slot
            sub_base = np.array([sub_off[bb] for bb in d["ub"]])
            esub = (sub_base + (0 if half == "lo" else NSL[d["ub"]]) + d["slot"] // 128)[d["inv"]]
            epart = (d["slot"] % 128)[d["inv"]]
            np.add.at(st, (epart, esub, d["dcol"]), d["nrm"])
        # self subtiles: diag entries
        for b in range(NBLK):
            s_idx = sub_off[b] + NS[b] - 1
            nvalid = min(128, R - b * 128)
            nodes = c * R + b * 128 + np.arange(nvalid)
            st[np.arange(nvalid), s_idx, np.arange(nvalid)] = self_norm[nodes]

        x_own = np.zeros((RPAD, D), dtype=BF16)
        x_own[:R] = xs_bf[c * R : (c + 1) * R]

        in_maps.append(
            {
                "xs_lo": xs_bf[:SPLIT],
                "xs_hi": xs_bf[SPLIT:],
                "x_own": x_own,
                "st": st.astype(BF16),
                "ix_lo": wrap(ix["lo"]),
                "ix_hi": wrap(ix["hi"]),
                "iz_lo": wrap(iz["lo"]),
                "iz_hi": wrap(iz["hi"]),
                "w1": w1_bf,
                "w2": w2_bf,
                "b1": b1_f,
                "b2bc": b2_bc,
                "eye": eye,
            }
        )
    return in_maps, meta


def build(meta):
    NSL, NSH, NS = meta["NSL"], meta["NSH"], meta["NS"]
    sub_off, TOT_SUB = meta["sub_off"], meta["TOT_SUB"]
    lo_boff, hi_boff = meta["lo_boff"], meta["hi_boff"]
    lo_glen, hi_glen = meta["lo_glen"], meta["hi_glen"]
    lo_ops, hi_ops = meta["lo_ops"], meta["hi_ops"]
    L_lo, L_hi = sum(lo_glen), sum(hi_glen)

    nc = bacc.Bacc("TRN2", target_bir_lowering=False, debug=False, num_devices=NCORES,
                   num_swdge_queues=4)
    f32, bf16, i16 = mybir.dt.float32, mybir.dt.bfloat16, mybir.dt.int16

    xs_lo = nc.declare_dram_parameter("xs_lo", [SPLIT, D], bf16, isOutput=False)
    xs_hi = nc.declare_dram_parameter("xs_hi", [N - SPLIT, D], bf16, isOutput=False)
    x_own = nc.declare_dram_parameter("x_own", [RPAD, D], bf16, isOutput=False)
    st_d = nc.declare_dram_parameter("st", [128, TOT_SUB, 128], bf16, isOutput=False)
    ix_lo = nc.declare_dram_parameter("ix_lo", [128, L_lo // 16], i16, isOutput=False)
    ix_hi = nc.declare_dram_parameter("ix_hi", [128, L_hi // 16], i16, isOutput=False)
    iz_lo = nc.declare_dram_parameter("iz_lo", [128, L_lo // 16], i16, isOutput=False)
    iz_hi = nc.declare_dram_parameter("iz_hi", [128, L_hi // 16], i16, isOutput=False)
    w1_d = nc.declare_dram_parameter("w1", [D, H], bf16, isOutput=False)
    w2_d = nc.declare_dram_parameter("w2", [H, D], bf16, isOutput=False)
    b1_d = nc.declare_dram_parameter("b1", [128, 4], f32, isOutput=False)
    b2_d = nc.declare_dram_parameter("b2bc", [128, 128], f32, isOutput=False)
    eye_d = nc.declare_dram_parameter("eye", [128, 128], bf16, isOutput=False)
    out_d = nc.declare_dram_parameter("out", [RPAD, D], f32, isOutput=True)

    z_own = nc.dram_tensor("z_own", [RPAD, D], bf16)
    z_full = nc.dram_tensor("z_full", [NCORES * RPAD, D], bf16, addr_space="Shared")

    with tile.TileContext(nc) as tc:
        with (
            tc.tile_pool(name="const", bufs=1) as cpool,
            tc.tile_pool(name="gpool", bufs=3) as gpool,
            tc.tile_pool(name="stp", bufs=3) as stpool,
            tc.tile_pool(name="small", bufs=3) as spool,
            tc.tile_pool(name="psA", bufs=2, space="PSUM") as psA,
            tc.tile_pool(name="psB", bufs=2, space="PSUM") as psB,
        ):
            w1_t = cpool.tile([128, H], bf16)
            nc.sync.dma_start(out=w1_t[:], in_=w1_d[:])
            w2_t = cpool.tile([128, 4, 128], bf16)
            nc.sync.dma_start(out=w2_t[:], in_=w2_d[:].rearrange("(m p) o -> p m o", p=128))
            b1_t = cpool.tile([128, 4], f32)
            nc.sync.dma_start(out=b1_t[:], in_=b1_d[:])
            b2_t = cpool.tile([128, 128], f32)
            nc.sync.dma_start(out=b2_t[:], in_=b2_d[:])
            eye_t = cpool.tile([128, 128], bf16)
            nc.sync.dma_start(out=eye_t[:], in_=eye_d[:])

            MAXSUB_LO = max(lo_glen) // 128
            MAXSUB_HI = max(hi_glen) // 128

            qn = [0]

            def do_phase(layer):
                if layer == 1:
                    src_lo, src_hi = xs_lo[:], xs_hi[:]
                    id_lo, id_hi = ix_lo, ix_hi
                    own_src = x_own
                else:
                    src_lo = z_full[0:ZSPLIT, :]
                    src_hi = z_full[ZSPLIT : NCORES * RPAD, :]
                    id_lo, id_hi = iz_lo, iz_hi
                    own_src = z_own
                lo_cum = 0
                hi_cum = 0
                for g in range(NGRP):
                    Llo, Lhi = lo_glen[g], hi_glen[g]
                    ilo = spool.tile([128, max(lo_glen) // 16], i16, tag="ilo")
                    nc.sync.dma_start(
                        out=ilo[:, : Llo // 16],
                        in_=id_lo[:, lo_cum // 16 : (lo_cum + Llo) // 16],
                    )
                    ihi = spool.tile([128, max(hi_glen) // 16], i16, tag="ihi")
                    nc.sync.dma_start(
                        out=ihi[:, : Lhi // 16],
                        in_=id_hi[:, hi_cum // 16 : (hi_cum + Lhi) // 16],
                    )
                    glo = gpool.tile([128, MAXSUB_LO, 128], bf16, tag="glo")
                    ghi = gpool.tile([128, MAXSUB_HI, 128], bf16, tag="ghi")
                    o = 0
                    for sz in lo_ops[g]:
                        nc.gpsimd.dma_gather(
                            glo[:, o // 128 : (o + sz) // 128, :], src_lo,
                            ilo[:, o // 16 : (o + sz) // 16], sz, sz, D,
                            queue_num=qn[0] % 4,
                        )
                        qn[0] += 1
                        o += sz
                    o = 0
                    for sz in hi_ops[g]:
                        nc.gpsimd.dma_gather(
                            ghi[:, o // 128 : (o + sz) // 128, :], src_hi,
                            ihi[:, o // 16 : (o + sz) // 16], sz, sz, D,
                            queue_num=qn[0] % 4,
                        )
                        qn[0] += 1
                        o += sz
                    lo_cum += Llo
                    hi_cum += Lhi

                    for br in range(GB):
                        b = g * GB + br
                        ns, nsl, nsh = int(NS[b]), int(NSL[b]), int(NSH[b])
                        nvalid = min(128, R - b * 128)
                        xo = spool.tile([128, 128], bf16, tag="xo")
                        nc.sync.dma_start(
                            out=xo[:nvalid, :],
                            in_=own_src[b * 128 : b * 128 + nvalid, :],
                        )
                        st_t = stpool.tile([128, int(NS.max()), 128], bf16, tag="st")
                        nc.sync.dma_start(
                            out=st_t[:, :ns, :],
                            in_=st_d[:, sub_off[b] : sub_off[b] + ns, :],
                        )
                        p = psA.tile([128, 128], f32, tag="p1")
                        for s in range(ns):
                            if s < nsl:
                                g_sl = glo[:, lo_boff[b] // 128 + s, :]
                            elif s < nsl + nsh:
                                g_sl = ghi[:, hi_boff[b] // 128 + (s - nsl), :]
                            else:
                                g_sl = xo[:]
                            if layer == 1:
                                nc.tensor.matmul(
                                    p[:], g_sl, st_t[:, s, :],
                                    start=(s == 0), stop=(s == ns - 1),
                                )
                            else:
                                nc.tensor.matmul(
                                    p[:], st_t[:, s, :], g_sl,
                                    start=(s == 0), stop=(s == ns - 1),
                                )
                        if layer == 1:
                            at = spool.tile([128, 128], bf16, tag="at")
                            nc.vector.tensor_copy(at[:], p[:])
                            hs = spool.tile([128, 4, 128], bf16, tag="hs")
                            for mi in range(4):
                                hp = psB.tile([128, 128], f32, tag="hp")
                                nc.tensor.matmul(
                                    hp[:], w1_t[:, mi * 128 : (mi + 1) * 128], at[:],
                                    start=True, stop=True,
                                )
                                nc.scalar.activation(
                                    hs[:, mi, :], hp[:],
                                    mybir.ActivationFunctionType.Relu,
                                    bias=b1_t[:, mi : mi + 1],
                                )
                            zp = psA.tile([128, 128], f32, tag="zp")
                            for mi in range(4):
                                nc.tensor.matmul(
                                    zp[:], w2_t[:, mi, :], hs[:, mi, :],
                                    start=(mi == 0), stop=(mi == 3),
                                )
                            zs = spool.tile([128, 128], bf16, tag="zs")
                            nc.vector.tensor_copy(zs[:], zp[:])
                            ztp = psB.tile([128, 128], bf16, tag="ztp")
                            nc.tensor.transpose(ztp[:], zs[:], eye_t[:])
                            zts = spool.tile([128, 128], bf16, tag="zts")
                            nc.vector.tensor_copy(zts[:], ztp[:])
                            nc.sync.dma_start(
                                out=z_own[b * 128 : (b + 1) * 128, :], in_=zts[:]
                            )
                        else:
                            ob = spool.tile([128, 128], f32, tag="ob")
                            nc.vector.tensor_tensor(
                                ob[:], p[:], b2_t[:], mybir.AluOpType.add
                            )
                            o2 = spool.tile([128, 128], f32, tag="o2")
                            nc.scalar.activation(
                                o2[:], ob[:], mybir.ActivationFunctionType.Relu
                            )
                            nc.sync.dma_start(
                                out=out_d[b * 128 : b * 128 + nvalid, :],
                                in_=o2[:nvalid, :],
                            )

            do_phase(1)
            nc.gpsimd.collective_compute(
                "AllGather",
                mybir.AluOpType.bypass,
                replica_groups=[list(range(NCORES))],
                ins=[z_own[:].opt()],
                outs=[z_full[:].opt()],
            )
            do_phase(2)

    nc.compile()
    return nc


_CACHE = {}


def kernel(x, edge_index, W1, b1, W2, b2):
    in_maps, meta = _prep(x, edge_index, W1, b1, W2, b2)
    key = (tuple(meta["NS"]), tuple(meta["lo_glen"]), tuple(meta["hi_glen"]))
    if key not in _CACHE:
        _CACHE[key] = build(meta)
    nc = _CACHE[key]
    res = run_bass_kernel_spmd(nc, in_maps, core_ids=list(range(NCORES)))
    out = np.concatenate([res.results[c]["out"][:R] for c in range(NCORES)], axis=0)
    return out.astype(np.float32)
